# revision 1
# baseline (speedup 1.0000x reference)
"""Trainium2 Bass kernel for MimiAttention (GQA + RoPE + causal softmax).

Problem: B=2, S=2048, H=1024, NH=16 q-heads, NKV=4 kv-heads, HD=64.
Sharding: 8 cores = 2 (batch) x 4 (kv-group).  Each core computes one batch's
attention for one GQA group (4 q-heads sharing 1 kv head) and the partial
o-projection for those heads; the host sums the 4 partials per batch.

Design (all matmuls bf16 with fp32 psum; ~134us cost-model time per core):
  * RoPE hat-trick: wqk columns carry [q; q2] per head (q2 = sign-permuted
    rows), qhat = proj * cs; khat = [k_rot; k_rot] via the J-fold matmul, so
    no partition-crossing vector ops are needed.
  * Scores computed transposed (scoresT[j, i]) one key-tile row at a time,
    streamed through two ping-pong [128, 1024] PSUM feed regions; ONE exp
    activation per <=1024-col segment (~96 exps total) minimizes the ACT
    engine's fixed per-instruction cost, which otherwise dominates.
  * Software pipeline: scores+exp for row r are issued before attnV of row
    r-1, so the in-order PE never stalls behind the exp/mask chain; the
    causal diagonal is masked in place on the exp output (DVE 2x-mode mult,
    Pool for head 3 where DVE is loaded).
  * attnV accumulates out[i, v|den] slices in 3 persistent PSUM banks
    (65-wide slices; column 64 = softmax denominator via a ones-column in
    v); per-slice normalization = DVE reciprocal + tensor_scalar.
  * attn[i,c] -> aT[c,i] via PE transposes batched 4-per-work-bank residency
    (pending-zero trick allows sub-bank packing); pair 0 during head 2,
    pair 1 inline during head 3.
  * o-projection streams during head 3 through the work bank and the
    progressively-freed attnV banks; the remainder runs after attention
    through 5 rotating psum slots with drains split DVE/ACT; output DMAs
    are batched 4 chunks (512 rows) each.
  * Input DMAs are ordered by first use (k weights, first xt column block,
    cs/fold tables, then column-major xt) so the k/q0 projections start
    ~4us in; producer work (v/k/q projections for later heads) is slotted
    at most one work-bank residency per row to avoid WAR thrash.
"""

import numpy as np
import ml_dtypes

B, S, H = 2, 2048, 1024
NH, NKV, HD = 16, 4, 64
G = NH // NKV            # 4 q-heads per kv head
THETA = 10000.0
N_CORES = 8

BF16 = ml_dtypes.bfloat16

NSB = S // 512           # 4 chunks of 512
NST = S // 128           # 16 tiles of 128
KC = H // 128            # 8 contraction chunks
SCALE = float(1.0 / np.sqrt(HD))


def _build_nc():
    import concourse.mybir as mybir
    import concourse.tile as tile
    from concourse.tile import add_dep_helper
    from concourse import bacc

    f32 = mybir.dt.float32
    bf16 = mybir.dt.bfloat16

    nc = bacc.Bacc("TRN2", target_bir_lowering=False)

    xTd = nc.dram_tensor("xT", [H, S], bf16, kind="ExternalInput")
    wqkd = nc.dram_tensor("wqkT", [H, 640], bf16, kind="ExternalInput")
    wk8d = nc.dram_tensor("wkT8", [128, KC, 128], bf16, kind="ExternalInput")
    wvd = nc.dram_tensor("wvT", [H, HD], bf16, kind="ExternalInput")
    csd = nc.dram_tensor("cs", [128, S], bf16, kind="ExternalInput")
    wod = nc.dram_tensor("woT", [G * HD, H], bf16, kind="ExternalInput")
    trid = nc.dram_tensor("trimask", [128, 128], bf16, kind="ExternalInput")
    djd = nc.dram_tensor("dupJ", [128, 128], bf16, kind="ExternalInput")
    idd = nc.dram_tensor("ident", [128, 128], bf16, kind="ExternalInput")
    oTd = nc.dram_tensor("oT", [H, S], bf16, kind="ExternalOutput")

    with tile.TileContext(nc) as tc:
        import contextlib
        ctx = contextlib.ExitStack()
        with ctx:
            consts = ctx.enter_context(tc.tile_pool(name="consts", bufs=1))
            acts = ctx.enter_context(tc.tile_pool(name="acts", bufs=1))
            ep = ctx.enter_context(tc.tile_pool(name="exps", bufs=3))
            rcp = ctx.enter_context(tc.tile_pool(name="rcp", bufs=6))
            otp = ctx.enter_context(tc.tile_pool(name="ot", bufs=4))
            pav = ctx.enter_context(
                tc.tile_pool(name="ps_av", bufs=1, space="PSUM"))
            pfa = ctx.enter_context(
                tc.tile_pool(name="ps_fa", bufs=1, space="PSUM"))
            pfb = ctx.enter_context(
                tc.tile_pool(name="ps_fb", bufs=1, space="PSUM"))
            pw = ctx.enter_context(
                tc.tile_pool(name="ps_w", bufs=1, space="PSUM"))

            # ---- input DMAs, ordered by first use: k weights + first xt
            # column block feed the k/q0 projections; the remaining xt lands
            # column-major so qhat chunks stream in order.
            xt_sb = consts.tile([128, KC, S], bf16, tag="xt")
            wqk_sb = consts.tile([128, KC, 640], bf16, tag="wqk")
            cs_sb = consts.tile([128, S], bf16, tag="cs")
            tri_sb = consts.tile([128, 128], bf16, tag="tri")
            dj_sb = consts.tile([128, 128], bf16, tag="dj")
            id_sb = consts.tile([128, 128], bf16, tag="id")
            wv_sb = consts.tile([128, KC, HD], bf16, tag="wv")
            wo_sb = consts.tile([128, 2, H], bf16, tag="wo")

            def xt_col(n):
                c = n * 512
                nc.sync.dma_start(
                    xt_sb[:, :, c:c + 512],
                    xTd[:, c:c + 512].rearrange("(kc p) m -> p kc m", p=128))

            def wqk_cols(c0, c1):
                nc.sync.dma_start(
                    wqk_sb[:, :, c0:c1],
                    wqkd[:, c0:c1].rearrange("(kc p) m -> p kc m", p=128))

            nc.sync.dma_start(wqk_sb[:, :, 512:640], wk8d[:, :, :])
            c = 0
            nc.sync.dma_start(
                xt_sb[:, 0:4, 0:512],
                xTd[0:512, 0:512].rearrange("(kc p) m -> p kc m", p=128))
            nc.sync.dma_start(
                xt_sb[:, 4:8, 0:512],
                xTd[512:1024, 0:512].rearrange("(kc p) m -> p kc m", p=128))
            nc.sync.dma_start(cs_sb, csd[:, :])
            nc.sync.dma_start(dj_sb, djd[:, :])
            wqk_cols(0, 128)            # q head 0
            xt_col(1)
            nc.sync.dma_start(wv_sb, wvd.rearrange("(kc p) m -> p kc m", p=128))
            xt_col(2)
            nc.sync.dma_start(tri_sb, trid[:, :])
            xt_col(3)
            wqk_cols(128, 512)          # q heads 1-3 (first used ~25us in)
            nc.sync.dma_start(wo_sb, wod.rearrange("(kc p) m -> p kc m", p=128))
            nc.sync.dma_start(id_sb, idd[:, :])

            qhat = [acts.tile([128, S], bf16, tag=f"qh{m}", name=f"qhat{m}")
                    for m in range(G)]
            khat = acts.tile([128, S], bf16, tag="khat")
            ktmp = acts.tile([128, S], bf16, tag="ktmp")
            v_sb = acts.tile([128, NST, HD + 1], bf16, tag="vsb")
            attn_n = acts.tile([128, NST, G * HD], bf16, tag="attn")
            aT = acts.tile([128, 2, S], bf16, tag="aT")

            avb = [pav.tile([128, w], f32, tag=f"av{b}", name=f"avb{b}")
                   for b, w in ((0, 455), (1, 455), (2, 130))]

            def av_slice(it):
                b, o = it // 7, (it % 7) * 65
                return avb[b][:, o:o + 65]

            seg_counter = [0]

            def feed_tile(idx, ln):
                # ping-pong exp-feed regions, allocated per segment so the
                # pool slot rotation provides the WAR chain
                if idx % 2 == 0:
                    return pfa.tile([128, ln], f32, tag="fA", name="feed",
                                    padded_shape=[128, 1024])
                return pfb.tile([128, ln], f32, tag="fB", name="feed",
                                padded_shape=[128, 1024])

            def proj_psum(m, n, ps):
                col = n * 512
                for kc in range(KC):
                    nc.tensor.matmul(
                        ps, wqk_sb[:, kc, m * 128:(m + 1) * 128],
                        xt_sb[:, kc, col:col + 512],
                        start=(kc == 0), stop=(kc == KC - 1))

            def q_chunk(h, n, ps=None):
                if ps is None:
                    ps = pw.tile([128, 512], f32, tag="w", name="psq")
                proj_psum(h, n, ps)
                col = n * 512
                nc.vector.tensor_mul(
                    qhat[h][:, col:col + 512], ps, cs_sb[:, col:col + 512])

            def k_proj(n, ps=None):
                if ps is None:
                    ps = pw.tile([128, 512], f32, tag="w", name="psk")
                proj_psum(G, n, ps)
                col = n * 512
                nc.vector.tensor_mul(
                    ktmp[:, col:col + 512], ps, cs_sb[:, col:col + 512])

            def k_fold(n, psf=None):
                col = n * 512
                if psf is None:
                    psf = pw.tile([128, 512], f32, tag="w", name="psf")
                nc.tensor.matmul(psf, dj_sb, ktmp[:, col:col + 512],
                                 start=True, stop=True)
                nc.vector.tensor_copy(khat[:, col:col + 512], psf)

            def k_chunk(n, ps=None, psf=None):
                k_proj(n, ps)
                k_fold(n, psf)

            def v_tiles(st0, nt):
                # project nt seq-tiles of v through one work-psum residency
                psv = pw.tile([128, nt, HD], f32, tag="w", name="psv",
                              padded_shape=[128, 4, HD])
                for t in range(nt):
                    st = st0 + t
                    for kc in range(KC):
                        nc.tensor.matmul(
                            psv[:, t, :],
                            xt_sb[:, kc, st * 128:(st + 1) * 128],
                            wv_sb[:, kc, :],
                            start=(t == 0 and kc == 0), stop=(kc == KC - 1),
                            skip_group_check=True)
                nc.vector.tensor_copy(
                    v_sb[:, st0:st0 + nt, 0:HD], psv)

            def transpose_group(hp, g4):
                # 4 slice transposes through one work-psum residency
                psx = pw.tile([128, 4, 128], bf16, tag="w", name="pst")
                for t in range(4):
                    it = g4 * 4 + t
                    nc.tensor.matmul(
                        psx[:, t, :], attn_n[:, it, hp * 128:(hp + 1) * 128],
                        id_sb, is_transpose=True,
                        start=(t == 0), stop=True, skip_group_check=True)
                nc.vector.tensor_copy(
                    aT[:, hp, g4 * 512:(g4 + 1) * 512], psx)

            # ---- prologue
            nc.gpsimd.memset(v_sb[:, :, HD:HD + 1], 1.0)
            k_proj(0, ps=feed_tile(0, 512))
            q_chunk(0, 0, ps=feed_tile(1, 512))
            k_fold(0, psf=pw.tile([128, 512], f32, tag="w", name="psf0"))
            q_chunk(0, 1, ps=pw.tile([128, 512], f32, tag="w", name="psq0"))
            v_tiles(0, 2)
            seg_counter[0] = 2

            def scores_row(h, jt, et, segs=None, cbs=None):
                lo = jt * 128
                cols = S - lo
                lhsT = khat[:, lo:lo + 128]
                if segs is None:
                    if cols > 1024:
                        segs = [(lo, cols - 1024), (lo + cols - 1024, 1024)]
                    else:
                        segs = [(lo, cols)]
                for si, (off, ln) in enumerate(segs):
                    region = feed_tile(seg_counter[0], ln)
                    seg_counter[0] += 1
                    done = 0
                    while done < ln:
                        cl = min(512, ln - done)
                        nc.tensor.matmul(
                            region[:, done:done + cl], lhsT,
                            qhat[h][:, off + done:off + done + cl],
                            start=True, stop=True)
                        done += cl
                    nc.scalar.activation(
                        et[:, off:off + ln], region[:, 0:ln],
                        mybir.ActivationFunctionType.Exp, scale=SCALE)
                    if cbs is not None and si in cbs:
                        cbs[si]()
                # causal mask on diag tile: Pool, hidden by the pipeline
                if h == 3:
                    nc.gpsimd.tensor_mul(et[:, lo:lo + 128],
                                         et[:, lo:lo + 128], tri_sb)
                else:
                    nc.vector.tensor_mul(et[:, lo:lo + 128],
                                         et[:, lo:lo + 128], tri_sb)

            attnv_state = {}   # h -> bank_first dict

            def attnv_row(h, jt, et):
                bank_first = attnv_state.setdefault(h, {})
                b1_hi = min(jt + 7, NST - 1)
                order = list(range(b1_hi, jt - 1, -1)) + \
                    list(range(NST - 1, b1_hi, -1))
                for it in order:
                    b = it // 7
                    first = jt == 0 and b not in bank_first
                    mm = nc.tensor.matmul(
                        av_slice(it), et[:, it * 128:(it + 1) * 128],
                        v_sb[:, jt, :],
                        start=first, stop=(it == jt),
                        skip_group_check=True)
                    if first:
                        bank_first[b] = mm
                    elif jt == 0:
                        add_dep_helper(mm.ins, bank_first[b].ins,
                                       sync=False,
                                       reason="bank clear first")
                pso = av_slice(jt)
                rc = rcp.tile([128, 1], f32, tag="rc", name="rc")
                nc.vector.reciprocal(rc, pso[:, HD:HD + 1])
                nc.vector.tensor_scalar_mul(
                    attn_n[:, jt, h * HD:(h + 1) * HD], pso[:, 0:HD], rc)

            # oproj -------------------------------------------------------
            oproj_pending = [(g, hc) for g in range(NSB) for hc in range(KC)]
            ot_state = {}

            def oproj_chunk(ps, drain_eng):
                g, hc = oproj_pending.pop(0)
                col = g * 512
                for kc2 in range(2):
                    nc.tensor.matmul(
                        ps, wo_sb[:, kc2, hc * 128:(hc + 1) * 128],
                        aT[:, kc2, col:col + 512],
                        start=(kc2 == 0), stop=(kc2 == 1))
                if hc % 4 == 0:
                    ot_state[g] = otp.tile([128, 4, 512], bf16, tag="otb",
                                           name="otb")
                ot = ot_state[g]
                if drain_eng == 0:
                    nc.vector.tensor_copy(ot[:, hc % 4, :], ps)
                elif drain_eng == 1:
                    nc.scalar.copy(ot[:, hc % 4, :], ps)
                else:
                    nc.vector.tensor_copy(ot[:, hc % 4, 0:256], ps[:, 0:256])
                    nc.scalar.copy(ot[:, hc % 4, 256:512], ps[:, 256:512])
                if hc % 4 == 3:
                    r0 = (hc // 4) * 512
                    nc.sync.dma_start(
                        oTd[r0:r0 + 512, col:col + 512].rearrange(
                            "(c p) m -> p c m", p=128), ot)

            # ---- main pipelined loop ------------------------------------
            seq = [(h, jt) for h in range(G) for jt in range(NST)]
            prev = None
            for (h, jt) in seq:
                et = ep.tile([128, S], bf16, tag="e", name=f"e{h}_{jt}")
                if h == 0 and jt == 0:
                    def _row0cb():
                        ps2 = feed_tile(seg_counter[0], 1024)
                        seg_counter[0] += 1
                        proj_psum(0, 2, ps2[:, 0:512])
                        proj_psum(0, 3, ps2[:, 512:1024])
                        nc.vector.tensor_mul(
                            qhat[0][:, 1024:2048], ps2, cs_sb[:, 1024:2048])
                    scores_row(h, jt, et, cbs={0: _row0cb})
                else:
                    scores_row(h, jt, et)
                if prev is not None:
                    attnv_row(*prev)
                prev = (h, jt, et)

                # interleaved producer work, at most ~one work-slot per row
                if h == 0:
                    if jt == 1:
                        v_tiles(2, 2)
                    if jt in (4, 8, 12):
                        v_tiles(jt + 0, 4)
                    if jt in (2, 6, 10):
                        k_proj(jt // 4 + 1)
                    if jt in (3, 7, 11):
                        k_fold((jt + 1) // 4)
                    if jt in (5, 9, 13, 15):
                        q_chunk(1, (5, 9, 13, 15).index(jt))
                if h in (1, 2) and jt in (1, 4, 7, 10):
                    q_chunk(h + 1, (jt - 1) // 3)
                if h == 2 and jt in (3, 7, 11, 15):
                    transpose_group(0, jt // 4)
                if h == 3:
                    if jt in (5, 9, 13):
                        transpose_group(1, (jt - 5) // 4)
                    if jt >= 5 and oproj_pending and \
                            oproj_pending[0][0] * 4 + 5 <= jt:
                        ps = pw.tile([128, 512], f32, tag="w", name="psow")
                        oproj_chunk(ps, drain_eng=0)
                    if jt >= 8 and oproj_pending and \
                            oproj_pending[0][0] * 4 + 5 <= jt:
                        ps = pav.tile([128, 512], f32, tag="av0",
                                      name="psoa")
                        oproj_chunk(ps, drain_eng=1 if jt >= 12 else 0)
                    if jt in (11, 13, 15) and oproj_pending and \
                            oproj_pending[0][0] * 4 + 5 <= jt:
                        ps = pav.tile([128, 512], f32, tag="av0",
                                      name="psoa2")
                        oproj_chunk(ps, drain_eng=1)
                    if jt in (14, 15) and oproj_pending and \
                            oproj_pending[0][0] * 4 + 5 <= jt:
                        ps = pav.tile([128, 512], f32, tag="av1",
                                      name="psob")
                        oproj_chunk(ps, drain_eng=1)

            # flush: last attnV row + final transposes + remaining oproj
            attnv_row(*prev)
            transpose_group(1, 3)
            ti = 0
            slots = ["fA", "fB", "av0", "w", "av1"]
            pools = {"w": pw, "fA": pfa, "fB": pfb, "av0": pav, "av1": pav}
            while oproj_pending:
                tag = slots[ti % len(slots)]
                ps = pools[tag].tile([128, 512], f32, tag=tag, name="psot")
                oproj_chunk(ps, drain_eng=ti % 2)
                ti += 1

    nc.finalize()
    return nc


def _host_inputs(hidden_states, position_ids, wq, wk, wv, wo):
    """Build the 8 per-core input maps."""
    def w2_of(w):
        # w: [64, H] rows of one head; returns sign-permuted rows
        w2 = np.empty_like(w)
        w2[:32] = -w[32:64]
        w2[32:] = w[:32]
        return w2

    trimask = np.triu(np.ones((128, 128), np.float32)).astype(BF16)
    dupJ = np.zeros((128, 128), np.float32)
    for p in range(128):
        dupJ[p, p % 64] = 1.0
        dupJ[p, p % 64 + 64] = 1.0
    dupJ = dupJ.astype(BF16)
    ident = np.eye(128, dtype=np.float32).astype(BF16)

    in_maps = []
    for core in range(N_CORES):
        b, kv = core // NKV, core % NKV
        xT = np.ascontiguousarray(hidden_states[b].T).astype(BF16)

        cols = []
        for i in range(G):
            h = kv * G + i
            wqh = wq[h * HD:(h + 1) * HD]
            cols.append(wqh.T)
            cols.append(w2_of(wqh).T)
        wkh = wk[kv * HD:(kv + 1) * HD]
        cols.append(wkh.T)
        cols.append(w2_of(wkh).T)
        wqkT = np.ascontiguousarray(np.concatenate(cols, axis=1)).astype(BF16)

        wvT = np.ascontiguousarray(wv[kv * HD:(kv + 1) * HD].T).astype(BF16)
        wkT8 = np.ascontiguousarray(
            wqkT[:, 512:640].reshape(KC, 128, 128).transpose(1, 0, 2)
        ).astype(BF16)
        woT = np.ascontiguousarray(
            wo[:, kv * G * HD:(kv + 1) * G * HD].T).astype(BF16)

        inv = 1.0 / (THETA ** (np.arange(0, HD, 2, dtype=np.float32) / HD))
        freqs = position_ids[b].astype(np.float32)[:, None] * inv[None, :]
        emb = np.concatenate([freqs, freqs], axis=-1)       # [S, 64]
        cs = np.concatenate([np.cos(emb).T, np.sin(emb).T], axis=0)  # [128, S]
        cs = np.ascontiguousarray(cs).astype(BF16)

        in_maps.append({
            "xT": xT, "wqkT": wqkT, "wvT": wvT, "cs": cs, "woT": woT,
            "trimask": trimask, "dupJ": dupJ, "ident": ident,
            "wkT8": wkT8,
        })
    return in_maps


_NC_CACHE = {}


def run_cores(in_maps, trace=False, trace_kwargs=None):
    from concourse.bass_utils import run_bass_kernel_spmd
    if "nc" not in _NC_CACHE:
        _NC_CACHE["nc"] = _build_nc()
    nc = _NC_CACHE["nc"]
    return run_bass_kernel_spmd(
        nc, in_maps, core_ids=list(range(N_CORES)),
        trace=trace, **(trace_kwargs or {}))


def kernel(hidden_states, attention_mask, position_ids, wq, wk, wv, wo):
    hidden_states = np.asarray(hidden_states, dtype=np.float32)
    position_ids = np.asarray(position_ids)
    wq = np.asarray(wq, dtype=np.float32)
    wk = np.asarray(wk, dtype=np.float32)
    wv = np.asarray(wv, dtype=np.float32)
    wo = np.asarray(wo, dtype=np.float32)

    in_maps = _host_inputs(hidden_states, position_ids, wq, wk, wv, wo)
    res = run_cores(in_maps)

    out = np.zeros((B, S, H), np.float32)
    for core in range(N_CORES):
        b = core // NKV
        out[b] += res.results[core]["oT"].T.astype(np.float32)
    return out



# revision 30
# speedup vs baseline: 1.0144x; 1.0144x over previous
"""Trainium2 Bass kernel for MimiAttention (GQA + RoPE + causal softmax).

Problem: B=2, S=2048, H=1024, NH=16 q-heads, NKV=4 kv-heads, HD=64.
Sharding: 8 cores = 2 (batch) x 4 (kv-group).  Each core computes one batch's
attention for one GQA group (4 q-heads sharing 1 kv head) and the partial
o-projection for those heads; the host sums the 4 partials per batch.

Design (all matmuls bf16 with fp32 psum; ~134us cost-model time per core):
  * RoPE hat-trick: wqk columns carry [q; q2] per head (q2 = sign-permuted
    rows), qhat = proj * cs; khat = [k_rot; k_rot] via the J-fold matmul, so
    no partition-crossing vector ops are needed.
  * Scores computed transposed (scoresT[j, i]) one key-tile row at a time,
    streamed through two ping-pong [128, 1024] PSUM feed regions; ONE exp
    activation per <=1024-col segment (~96 exps total) minimizes the ACT
    engine's fixed per-instruction cost, which otherwise dominates.
  * Software pipeline: scores+exp for row r are issued before attnV of row
    r-1, so the in-order PE never stalls behind the exp/mask chain; the
    causal diagonal is masked in place on the exp output (DVE 2x-mode mult,
    Pool for head 3 where DVE is loaded).
  * attnV accumulates out[i, v|den] slices in 3 persistent PSUM banks
    (65-wide slices; column 64 = softmax denominator via a ones-column in
    v); per-slice normalization = DVE reciprocal + tensor_scalar.
  * attn[i,c] -> aT[c,i] via PE transposes batched 4-per-work-bank residency
    (pending-zero trick allows sub-bank packing); pair 0 during head 2,
    pair 1 inline during head 3.
  * o-projection streams during head 3 through the work bank and the
    progressively-freed attnV banks; the remainder runs after attention
    through 5 rotating psum slots with drains split DVE/ACT; output DMAs
    are batched 4 chunks (512 rows) each.
  * Input DMAs are ordered by first use (k weights, first xt column block,
    cs/fold tables, then column-major xt) so the k/q0 projections start
    ~4us in; producer work (v/k/q projections for later heads) is slotted
    at most one work-bank residency per row to avoid WAR thrash.
"""

import numpy as np
import ml_dtypes

B, S, H = 2, 2048, 1024
NH, NKV, HD = 16, 4, 64
G = NH // NKV            # 4 q-heads per kv head
THETA = 10000.0
N_CORES = 8

BF16 = ml_dtypes.bfloat16

NSB = S // 512           # 4 chunks of 512
NST = S // 128           # 16 tiles of 128
KC = H // 128            # 8 contraction chunks
SCALE = float(1.0 / np.sqrt(HD))
N_WARM = 40              # PE p-state warmup matmuls (128 cols each)


def _build_nc():
    import concourse.mybir as mybir
    import concourse.tile as tile
    from concourse.tile import add_dep_helper
    from concourse import bacc

    f32 = mybir.dt.float32
    bf16 = mybir.dt.bfloat16

    nc = bacc.Bacc("TRN2", target_bir_lowering=False)

    xTd = nc.dram_tensor("xT", [H, S], bf16, kind="ExternalInput")
    wqkd = nc.dram_tensor("wqkT", [H, 640], bf16, kind="ExternalInput")
    wk8d = nc.dram_tensor("wkT8", [128, KC * 128], bf16, kind="ExternalInput")
    wq08d = nc.dram_tensor("wq0T8", [128, KC * 128], bf16,
                           kind="ExternalInput")
    wvd = nc.dram_tensor("wvT", [H, HD], bf16, kind="ExternalInput")
    csd = nc.dram_tensor("cs", [128, S], bf16, kind="ExternalInput")
    wod = nc.dram_tensor("woT", [G * HD, H], bf16, kind="ExternalInput")
    trid = nc.dram_tensor("trimask", [128, 128], bf16, kind="ExternalInput")
    djd = nc.dram_tensor("dupJ", [128, 128], bf16, kind="ExternalInput")
    idd = nc.dram_tensor("ident", [128, 128], bf16, kind="ExternalInput")
    # outputs: main seq cols 0:1536 as [H, 1536] bf16; seq 1536:2048 flat
    # bf16, per query tile ([feat%128, tile, half, hcg, s]) so each of the
    # last 4 query tiles streams out as soon as its attnV row completes.
    # The host reassembles.
    oTd = nc.dram_tensor("oT", [H, 3 * 512], bf16, kind="ExternalOutput")
    oT3d = nc.dram_tensor("oT3", [128, 2 * 4 * 384], bf16,
                          kind="ExternalOutput")
    oT2d = nc.dram_tensor("oT2", [128, 8 * 128], bf16, kind="ExternalOutput")

    with tile.TileContext(nc) as tc:
        import contextlib
        ctx = contextlib.ExitStack()
        with ctx:
            consts = ctx.enter_context(tc.tile_pool(name="consts", bufs=1))
            acts = ctx.enter_context(tc.tile_pool(name="acts", bufs=1))
            ep = ctx.enter_context(tc.tile_pool(name="exps", bufs=3))
            rcp = ctx.enter_context(tc.tile_pool(name="rcp", bufs=6))
            otp = ctx.enter_context(tc.tile_pool(name="ot", bufs=8))
            pav = ctx.enter_context(
                tc.tile_pool(name="ps_av", bufs=1, space="PSUM"))
            pfa = ctx.enter_context(
                tc.tile_pool(name="ps_fa", bufs=1, space="PSUM"))
            pfb = ctx.enter_context(
                tc.tile_pool(name="ps_fb", bufs=1, space="PSUM"))
            pw = ctx.enter_context(
                tc.tile_pool(name="ps_w", bufs=1, space="PSUM"))

            # ---- input DMAs, ordered by first use: k weights + first xt
            # column block feed the k/q0 projections; the remaining xt lands
            # column-major so qhat chunks stream in order.  wk/wq0 use
            # host-preswizzled contiguous [128, KC*128] layouts so their DMA
            # descriptors are 2KB (no sub-512B penalty), and the first xt
            # column block is split into kc-pair chunks so the k projection
            # can start after ~1/4 of it has landed.
            xt_sb = consts.tile([128, KC, S], bf16, tag="xt")
            wqk_sb = consts.tile([128, KC, 640], bf16, tag="wqk")
            wk_sb = consts.tile([128, KC * 128], bf16, tag="wk")
            wq0_sb = consts.tile([128, KC * 128], bf16, tag="wq0")
            cs_sb = consts.tile([128, S], bf16, tag="cs")
            tri_sb = consts.tile([128, 128], bf16, tag="tri")
            dj_sb = consts.tile([128, 128], bf16, tag="dj")
            id_sb = consts.tile([128, 128], bf16, tag="id")
            wv_sb = consts.tile([128, KC, HD], bf16, tag="wv")
            wo_sb = consts.tile([128, 2, H], bf16, tag="wo")
            warm_sb = consts.tile([128, 128], bf16, tag="warm")

            # PE warmup: burn the p-state ramp against a memset tile while
            # the first input DMAs are in flight.
            nc.gpsimd.memset(warm_sb, 0.0)
            warm_ps = pw.tile([128, 128], f32, tag="w", name="warmps")
            for _ in range(N_WARM):
                nc.tensor.matmul(warm_ps, warm_sb, warm_sb,
                                 start=True, stop=True, skip_group_check=True)

            def xt_col(n):
                c = n * 512
                nc.sync.dma_start(
                    xt_sb[:, :, c:c + 512],
                    xTd[:, c:c + 512].rearrange("(kc p) m -> p kc m", p=128))

            def wqk_cols(c0, c1):
                nc.sync.dma_start(
                    wqk_sb[:, :, c0:c1],
                    wqkd[:, c0:c1].rearrange("(kc p) m -> p kc m", p=128))

            nc.sync.dma_start(wk_sb, wk8d[:, :])
            for kk in range(4):
                r0 = kk * 256
                nc.sync.dma_start(
                    xt_sb[:, 2 * kk:2 * kk + 2, 0:512],
                    xTd[r0:r0 + 256, 0:512].rearrange(
                        "(kc p) m -> p kc m", p=128))
                if kk == 1:
                    nc.sync.dma_start(cs_sb[:, 0:1024], csd[:, 0:1024])
            nc.sync.dma_start(dj_sb, djd[:, :])
            nc.sync.dma_start(wq0_sb, wq08d[:, :])
            nc.sync.dma_start(cs_sb[:, 1024:2048], csd[:, 1024:2048])
            xt_col(1)
            nc.sync.dma_start(wv_sb, wvd.rearrange("(kc p) m -> p kc m", p=128))
            xt_col(2)
            nc.sync.dma_start(tri_sb, trid[:, :])
            xt_col(3)
            wqk_cols(128, 512)          # q heads 1-3 (first used ~25us in)
            nc.sync.dma_start(wo_sb, wod.rearrange("(kc p) m -> p kc m", p=128))
            nc.sync.dma_start(id_sb, idd[:, :])

            qhat = [acts.tile([128, S], bf16, tag=f"qh{m}", name=f"qhat{m}")
                    for m in range(G)]
            khat = acts.tile([128, S], bf16, tag="khat")
            ktmp = acts.tile([128, S], bf16, tag="ktmp")
            v_sb = acts.tile([128, NST, HD + 1], bf16, tag="vsb")
            attn_n = acts.tile([128, NST, G * HD], bf16, tag="attn")
            aT = acts.tile([128, 2, S], bf16, tag="aT")

            avb = [pav.tile([128, w], f32, tag=f"av{b}", name=f"avb{b}")
                   for b, w in ((0, 455), (1, 455), (2, 130))]

            def av_slice(it):
                b, o = it // 7, (it % 7) * 65
                return avb[b][:, o:o + 65]

            seg_counter = [0]

            def feed_tile(idx, ln):
                # ping-pong exp-feed regions, allocated per segment so the
                # pool slot rotation provides the WAR chain
                if idx % 2 == 0:
                    return pfa.tile([128, ln], f32, tag="fA", name="feed",
                                    padded_shape=[128, 1024])
                return pfb.tile([128, ln], f32, tag="fB", name="feed",
                                padded_shape=[128, 1024])

            def proj_lhs(m, kc):
                if m == 0:
                    return wq0_sb[:, kc * 128:(kc + 1) * 128]
                if m == G:
                    return wk_sb[:, kc * 128:(kc + 1) * 128]
                return wqk_sb[:, kc, m * 128:(m + 1) * 128]

            def proj_psum(m, n, ps):
                col = n * 512
                for kc in range(KC):
                    nc.tensor.matmul(
                        ps, proj_lhs(m, kc),
                        xt_sb[:, kc, col:col + 512],
                        start=(kc == 0), stop=(kc == KC - 1))

            def q_chunk(h, n, ps=None):
                if ps is None:
                    ps = pw.tile([128, 512], f32, tag="w", name="psq")
                proj_psum(h, n, ps)
                col = n * 512
                nc.vector.tensor_mul(
                    qhat[h][:, col:col + 512], ps, cs_sb[:, col:col + 512])

            def k_proj(n, ps=None):
                if ps is None:
                    ps = pw.tile([128, 512], f32, tag="w", name="psk")
                proj_psum(G, n, ps)
                col = n * 512
                nc.vector.tensor_mul(
                    ktmp[:, col:col + 512], ps, cs_sb[:, col:col + 512])

            def k_fold(n, psf=None):
                col = n * 512
                if psf is None:
                    psf = pw.tile([128, 512], f32, tag="w", name="psf")
                nc.tensor.matmul(psf, dj_sb, ktmp[:, col:col + 512],
                                 start=True, stop=True)
                nc.vector.tensor_copy(khat[:, col:col + 512], psf)

            def k_chunk(n, ps=None, psf=None):
                k_proj(n, ps)
                k_fold(n, psf)

            def v_tiles(st0, nt):
                # project nt seq-tiles of v through one work-psum residency
                psv = pw.tile([128, nt, HD], f32, tag="w", name="psv",
                              padded_shape=[128, 4, HD])
                for t in range(nt):
                    st = st0 + t
                    for kc in range(KC):
                        nc.tensor.matmul(
                            psv[:, t, :],
                            xt_sb[:, kc, st * 128:(st + 1) * 128],
                            wv_sb[:, kc, :],
                            start=(t == 0 and kc == 0), stop=(kc == KC - 1),
                            skip_group_check=True)
                nc.vector.tensor_copy(
                    v_sb[:, st0:st0 + nt, 0:HD], psv)

            def transpose_tiles(hp, its):
                # slice transposes through one work-psum residency
                psx = pw.tile([128, len(its), 128], bf16, tag="w", name="pst",
                              padded_shape=[128, 4, 128])
                for t, it in enumerate(its):
                    nc.tensor.matmul(
                        psx[:, t, :], attn_n[:, it, hp * 128:(hp + 1) * 128],
                        id_sb, is_transpose=True,
                        start=(t == 0), stop=True, skip_group_check=True)
                c0 = its[0] * 128
                nc.vector.tensor_copy(
                    aT[:, hp, c0:c0 + len(its) * 128], psx)

            def transpose_group(hp, g4):
                transpose_tiles(hp, list(range(g4 * 4, g4 * 4 + 4)))

            # ---- prologue
            nc.gpsimd.memset(v_sb[:, :, HD:HD + 1], 1.0)
            k_proj(0, ps=feed_tile(0, 512))
            q_chunk(0, 0, ps=feed_tile(1, 512))
            k_fold(0, psf=pw.tile([128, 512], f32, tag="w", name="psf0"))
            q_chunk(0, 1, ps=pw.tile([128, 512], f32, tag="w", name="psq0"))
            v_tiles(0, 2)
            seg_counter[0] = 2

            def scores_row(h, jt, et, segs=None, cbs=None):
                lo = jt * 128
                cols = S - lo
                lhsT = khat[:, lo:lo + 128]
                if segs is None:
                    if cols > 1024:
                        segs = [(lo, cols - 1024), (lo + cols - 1024, 1024)]
                    else:
                        segs = [(lo, cols)]
                for si, (off, ln) in enumerate(segs):
                    region = feed_tile(seg_counter[0], ln)
                    seg_counter[0] += 1
                    done = 0
                    while done < ln:
                        cl = min(512, ln - done)
                        nc.tensor.matmul(
                            region[:, done:done + cl], lhsT,
                            qhat[h][:, off + done:off + done + cl],
                            start=True, stop=True)
                        done += cl
                    nc.scalar.activation(
                        et[:, off:off + ln], region[:, 0:ln],
                        mybir.ActivationFunctionType.Exp, scale=SCALE)
                    if cbs is not None and si in cbs:
                        cbs[si]()
                # causal mask on diag tile: Pool, hidden by the pipeline
                if h == 3:
                    nc.gpsimd.tensor_mul(et[:, lo:lo + 128],
                                         et[:, lo:lo + 128], tri_sb)
                else:
                    nc.vector.tensor_mul(et[:, lo:lo + 128],
                                         et[:, lo:lo + 128], tri_sb)

            attnv_state = {}   # h -> bank_first dict

            def attnv_row(h, jt, et):
                bank_first = attnv_state.setdefault(h, {})
                b1_hi = min(jt + 7, NST - 1)
                order = list(range(b1_hi, jt - 1, -1)) + \
                    list(range(NST - 1, b1_hi, -1))
                for it in order:
                    b = it // 7
                    first = jt == 0 and b not in bank_first
                    mm = nc.tensor.matmul(
                        av_slice(it), et[:, it * 128:(it + 1) * 128],
                        v_sb[:, jt, :],
                        start=first, stop=(it == jt),
                        skip_group_check=True)
                    if first:
                        bank_first[b] = mm
                    elif jt == 0:
                        add_dep_helper(mm.ins, bank_first[b].ins,
                                       sync=False,
                                       reason="bank clear first")
                pso = av_slice(jt)
                rc = rcp.tile([128, 1], f32, tag="rc", name="rc")
                nc.vector.reciprocal(rc, pso[:, HD:HD + 1])
                nc.vector.tensor_scalar_mul(
                    attn_n[:, jt, h * HD:(h + 1) * HD], pso[:, 0:HD], rc)

            # oproj -------------------------------------------------------
            # Column groups g=0..2 keep the original 4-hc-batched [512,512]
            # output DMAs (few HWDGE entries).  Group 3 (seq 1536:2048) is
            # split: a 384-wide part (query tiles 12-14, ready one attnV row
            # before the end) drained + DMA'd flat, and a final 128-wide
            # sliver (tile 15) that is DMA'd directly from PSUM as f32 so
            # the kernel tail is one small transfer with no drain wait.
            oproj_pending = [(g, hc) for g in range(3) for hc in range(KC)]
            ot_state = {}

            def oproj_chunk(ps, drain_eng):
                g, hc = oproj_pending.pop(0)
                col = g * 512
                for kc2 in range(2):
                    nc.tensor.matmul(
                        ps, wo_sb[:, kc2, hc * 128:(hc + 1) * 128],
                        aT[:, kc2, col:col + 512],
                        start=(kc2 == 0), stop=(kc2 == 1))
                if hc % 4 == 0:
                    ot_state[g] = otp.tile([128, 4, 512], bf16, tag="otb",
                                           name="otb")
                ot = ot_state[g]
                if drain_eng == 0:
                    nc.vector.tensor_copy(ot[:, hc % 4, :], ps)
                elif drain_eng == 1:
                    nc.scalar.copy(ot[:, hc % 4, :], ps)
                else:
                    nc.vector.tensor_copy(ot[:, hc % 4, 0:256], ps[:, 0:256])
                    nc.scalar.copy(ot[:, hc % 4, 256:512], ps[:, 256:512])
                if hc % 4 == 3:
                    r0 = (hc // 4) * 512
                    nc.sync.dma_start(
                        oTd[r0:r0 + 512, col:col + 512].rearrange(
                            "(c p) m -> p c m", p=128), ot)

            g3_pending = list(range(KC))
            ot3_state = {}

            def g3_chunk(ps, drain_eng):
                hc = g3_pending.pop(0)
                for kc2 in range(2):
                    nc.tensor.matmul(
                        ps[:, 0:384], wo_sb[:, kc2, hc * 128:(hc + 1) * 128],
                        aT[:, kc2, 1536:1920],
                        start=(kc2 == 0), stop=(kc2 == 1))
                if hc % 4 == 0:
                    ot3_state[hc // 4] = otp.tile(
                        [128, 4, 384], bf16, tag="ot3", name="ot3")
                ot = ot3_state[hc // 4]
                if drain_eng == 0:
                    nc.vector.tensor_copy(ot[:, hc % 4, :], ps[:, 0:384])
                else:
                    nc.scalar.copy(ot[:, hc % 4, :], ps[:, 0:384])
                if hc % 4 == 3:
                    grp = hc // 4
                    nc.sync.dma_start(
                        oT3d[:, grp * 1536:(grp + 1) * 1536], ot)

            # ---- main pipelined loop ------------------------------------
            seq = [(h, jt) for h in range(G) for jt in range(NST)]
            prev = None
            for (h, jt) in seq:
                et = ep.tile([128, S], bf16, tag="e", name=f"e{h}_{jt}")
                if h == 0 and jt == 0:
                    def _row0cb():
                        ps2 = feed_tile(seg_counter[0], 1024)
                        seg_counter[0] += 1
                        proj_psum(0, 2, ps2[:, 0:512])
                        proj_psum(0, 3, ps2[:, 512:1024])
                        nc.vector.tensor_mul(
                            qhat[0][:, 1024:2048], ps2, cs_sb[:, 1024:2048])
                    scores_row(h, jt, et, cbs={0: _row0cb})
                else:
                    scores_row(h, jt, et)
                if prev is not None:
                    attnv_row(*prev)
                prev = (h, jt, et)

                # interleaved producer work, at most ~one work-slot per row
                if h == 0:
                    if jt == 1:
                        v_tiles(2, 2)
                    if jt in (4, 8, 12):
                        v_tiles(jt + 0, 4)
                    if jt in (2, 6, 10):
                        k_proj(jt // 4 + 1)
                    if jt in (3, 7, 11):
                        k_fold((jt + 1) // 4)
                    if jt in (5, 9, 13, 15):
                        q_chunk(1, (5, 9, 13, 15).index(jt))
                if h in (1, 2) and jt in (1, 4, 7, 10):
                    q_chunk(h + 1, (jt - 1) // 3)
                if h == 2 and jt in (3, 7, 11, 15):
                    transpose_group(0, jt // 4)
                if h == 3:
                    if jt in (5, 9, 13):
                        transpose_group(1, (jt - 5) // 4)
                    if jt >= 5 and oproj_pending and \
                            oproj_pending[0][0] * 4 + 5 <= jt:
                        ps = pw.tile([128, 512], f32, tag="w", name="psow")
                        oproj_chunk(ps, drain_eng=0)
                    if jt >= 8 and oproj_pending and \
                            oproj_pending[0][0] * 4 + 5 <= jt:
                        ps = pav.tile([128, 512], f32, tag="av0",
                                      name="psoa")
                        oproj_chunk(ps, drain_eng=1 if jt >= 12 else 0)
                    if jt in (11, 13, 15) and oproj_pending and \
                            oproj_pending[0][0] * 4 + 5 <= jt:
                        ps = pav.tile([128, 512], f32, tag="av0",
                                      name="psoa2")
                        oproj_chunk(ps, drain_eng=1)
                    if jt in (14, 15) and oproj_pending and \
                            oproj_pending[0][0] * 4 + 5 <= jt:
                        ps = pav.tile([128, 512], f32, tag="av1",
                                      name="psob")
                        oproj_chunk(ps, drain_eng=1)
                    if jt == 15:
                        # rows 12-14 of head 3 are normalized; pair-1
                        # transposes for tiles 12-14 unblock the 384-wide
                        # part of column group 3.
                        transpose_tiles(1, [12, 13, 14])

            # flush: last attnV row, then the 384-wide part of group 3
            # (query tiles 12-14), the tile-15 transpose, and the sliver.
            attnv_row(*prev)
            ti = 0
            slots = ["fA", "fB", "av0", "w", "av1"]
            pools = {"w": pw, "fA": pfa, "fB": pfb, "av0": pav, "av1": pav}
            while oproj_pending:
                tag = slots[ti % len(slots)]
                ps = pools[tag].tile([128, 512], f32, tag=tag, name="psot")
                oproj_chunk(ps, drain_eng=ti % 2)
                ti += 1

            def g3_next(drain_eng):
                tag = slots[ti % len(slots)]
                ps = pools[tag].tile([128, 512], f32, tag=tag, name="psog3")
                g3_chunk(ps, drain_eng)

            # two g3 chunks cover the normalize latency of row 15, then the
            # tile-15 transpose slots in, then the rest.
            g3_next(0)
            g3_next(1)
            ti += 2
            transpose_tiles(1, [15])
            while g3_pending:
                g3_next(ti % 2)
                ti += 1
            del g3_next

            # sliver: 8 feature chunks x 128 seq cols; two [128, 512] f32
            # psum tiles, drained on parallel engines, two small flat DMAs.
            for half in range(2):
                psl = (pfa if half == 0 else pfb).tile(
                    [128, 512], f32, tag=("fA" if half == 0 else "fB"),
                    name="psliv", padded_shape=[128, 1024])
                for sub in range(4):
                    hc = half * 4 + sub
                    for kc2 in range(2):
                        nc.tensor.matmul(
                            psl[:, sub * 128:(sub + 1) * 128],
                            wo_sb[:, kc2, hc * 128:(hc + 1) * 128],
                            aT[:, kc2, 1920:2048],
                            start=(kc2 == 0), stop=(kc2 == 1))
                ot2 = otp.tile([128, 512], bf16, tag="ot2", name="ot2")
                if half == 0:
                    nc.vector.tensor_copy(ot2, psl)
                else:
                    nc.scalar.copy(ot2, psl)
                nc.sync.dma_start(
                    oT2d[:, half * 512:(half + 1) * 512], ot2)

    nc.finalize()
    return nc


def _host_inputs(hidden_states, position_ids, wq, wk, wv, wo):
    """Build the 8 per-core input maps."""
    def w2_of(w):
        # w: [64, H] rows of one head; returns sign-permuted rows
        w2 = np.empty_like(w)
        w2[:32] = -w[32:64]
        w2[32:] = w[:32]
        return w2

    trimask = np.triu(np.ones((128, 128), np.float32)).astype(BF16)
    dupJ = np.zeros((128, 128), np.float32)
    for p in range(128):
        dupJ[p, p % 64] = 1.0
        dupJ[p, p % 64 + 64] = 1.0
    dupJ = dupJ.astype(BF16)
    ident = np.eye(128, dtype=np.float32).astype(BF16)

    in_maps = []
    for core in range(N_CORES):
        b, kv = core // NKV, core % NKV
        xT = np.ascontiguousarray(hidden_states[b].T).astype(BF16)

        cols = []
        for i in range(G):
            h = kv * G + i
            wqh = wq[h * HD:(h + 1) * HD]
            cols.append(wqh.T)
            cols.append(w2_of(wqh).T)
        wkh = wk[kv * HD:(kv + 1) * HD]
        cols.append(wkh.T)
        cols.append(w2_of(wkh).T)
        wqkT = np.ascontiguousarray(np.concatenate(cols, axis=1)).astype(BF16)

        wvT = np.ascontiguousarray(wv[kv * HD:(kv + 1) * HD].T).astype(BF16)
        wkT8 = np.ascontiguousarray(
            wqkT[:, 512:640].reshape(KC, 128, 128).transpose(1, 0, 2)
            .reshape(128, KC * 128)).astype(BF16)
        wq0T8 = np.ascontiguousarray(
            wqkT[:, 0:128].reshape(KC, 128, 128).transpose(1, 0, 2)
            .reshape(128, KC * 128)).astype(BF16)
        woT = np.ascontiguousarray(
            wo[:, kv * G * HD:(kv + 1) * G * HD].T).astype(BF16)

        inv = 1.0 / (THETA ** (np.arange(0, HD, 2, dtype=np.float32) / HD))
        freqs = position_ids[b].astype(np.float32)[:, None] * inv[None, :]
        emb = np.concatenate([freqs, freqs], axis=-1)       # [S, 64]
        cs = np.concatenate([np.cos(emb).T, np.sin(emb).T], axis=0)  # [128, S]
        cs = np.ascontiguousarray(cs).astype(BF16)

        in_maps.append({
            "xT": xT, "wqkT": wqkT, "wvT": wvT, "cs": cs, "woT": woT,
            "trimask": trimask, "dupJ": dupJ, "ident": ident,
            "wkT8": wkT8, "wq0T8": wq0T8,
        })
    return in_maps


_NC_CACHE = {}


def run_cores(in_maps, trace=False, trace_kwargs=None):
    from concourse.bass_utils import run_bass_kernel_spmd
    if "nc" not in _NC_CACHE:
        _NC_CACHE["nc"] = _build_nc()
    nc = _NC_CACHE["nc"]
    return run_bass_kernel_spmd(
        nc, in_maps, core_ids=list(range(N_CORES)),
        trace=trace, **(trace_kwargs or {}))


def assemble(res):
    """Gather the 3 per-core output pieces into the full [B, S, H] output."""
    out = np.zeros((B, S, H), np.float32)
    for core in range(N_CORES):
        b = core // NKV
        r = res.results[core]
        out[b, 0:1536] += r["oT"].T.astype(np.float32)
        out[b, 1536:1920] += (
            r["oT3"].reshape(128, 2, 4, 384).transpose(3, 1, 2, 0)
            .reshape(384, H).astype(np.float32))
        out[b, 1920:2048] += (
            r["oT2"].reshape(128, 8, 128).transpose(2, 1, 0)
            .reshape(128, H).astype(np.float32))
    return out


def kernel(hidden_states, attention_mask, position_ids, wq, wk, wv, wo):
    hidden_states = np.asarray(hidden_states, dtype=np.float32)
    position_ids = np.asarray(position_ids)
    wq = np.asarray(wq, dtype=np.float32)
    wk = np.asarray(wk, dtype=np.float32)
    wv = np.asarray(wv, dtype=np.float32)
    wo = np.asarray(wo, dtype=np.float32)

    in_maps = _host_inputs(hidden_states, position_ids, wq, wk, wv, wo)
    res = run_cores(in_maps)
    return assemble(res)

